# revision 3
# baseline (speedup 1.0000x reference)
"""Trainium2 Bass kernel for nn_EulerAttentionVariant (causal Euler attention).

Sharding: 32 (batch, head) pairs across 8 cores, 4 pairs/core (SPMD).

v2 design (vs the 83us baseline):
- QK matmuls in fp8-e4m3 DoubleRow perf mode (0.5 PE cycles/score column):
  host ships Q~/K~ as [64 features, 2(cos|sin), S] fp8, two pairs packed per
  128-partition tile (base_partition 0/64).
- Causal diagonal masking done ON THE PE: a constant tri(-448) x Id fp8
  DoubleRow matmul accumulates -448 into the masked (s < t) half of each
  diagonal score block before the exp, so exp(masked) ~ e^-28 ~ 0.  No
  mask work on ACT/DVE/Pool at all.
- The S^2/2 exp stream is split across THREE engines per score group:
  ACT runs true Exp; DVE and Pool run a Schraudolph bf16 exp
  (int16(x*184.665/scale + 16256) bit-viewed as bf16, ~0.1% rel err which
  softmax normalization mostly cancels).  All three write slices of one
  int16 PT tile that the PE consumes bitcast as bf16.
- PV runs TRANSPOSED: out[s-chunk 128, 65] = PT_block^T @ [V | 1], 65
  moving columns per 128x128 score block (half the PE cycles of the
  [65, s] orientation; output lands in natural [s, feature] order).
- No on-device softmax normalization: the unnormalized numerator and
  denominator stream out in bf16; the host divides and applies the final
  sqrt(2)*sin(u/(1+|w_out|)+b_out+pi/4) epilogue during the gather.
"""
import sys, os, math

for _p in ("/opt/trn_rl_repo", "/root/.axon_site/_ro/trn_rl_repo"):
    if os.path.isdir(_p) and _p not in sys.path:
        sys.path.insert(0, _p)

import numpy as np
import ml_dtypes
import concourse.bass as bass
import concourse.mybir as mybir
import concourse.tile as tile
from concourse import bacc
from concourse.bass_utils import run_bass_kernel_spmd

F32 = mybir.dt.float32
BF16 = mybir.dt.bfloat16
I16 = mybir.dt.int16
F8E4 = mybir.dt.float8e4
F8E5 = mybir.dt.float8e5
AF = mybir.ActivationFunctionType
OP = mybir.AluOpType
DR = mybir.MatmulPerfMode.DoubleRow

PI = math.pi
PHI = (1.0 + math.sqrt(5.0)) / 2.0
B, S, D, H = 2, 2048, 1024, 16
DH = D // H            # 64
NP = 4                 # pairs per core
NT = S // 128          # 16 k-tiles
NC = S // 128          # 16 s-chunks
SCALE = math.sqrt(2.0 * DH)   # sqrt(128)
BF = ml_dtypes.bfloat16
F8 = ml_dtypes.float8_e4m3
F8W = ml_dtypes.float8_e5m2        # wide-range fp8 for the mask constants

MASKC = -768.0                     # raw-score offset for masked (s<t) entries
A_SCH = (2.0 ** 7 / math.log(2.0)) / SCALE
B_SCH = float(127 * 2 ** 7)        # 16256

GB = 11                 # 128-col blocks per score group
# exp BLOCK split per group (ACT true exp / DVE / Pool schraudolph).
# Each engine gets its OWN single-bank score tile and its own PT tile:
# sharing either would serialize the engines through the tile
# framework's reader/writer chaining.
BLK_ACT = 4
BLK_DVE = 4

_CACHE = {}


def _blocks_of(h):
    """(k-tile ii, global s-chunk c) blocks of half h, ii-major."""
    out = []
    for ii in range(8 * h + 8):
        for c in range(max(ii, 8 * h), 8 * h + 8):
            out.append((ii, c))
    return out


def _build_nc():
    nc = bacc.Bacc("TRN2")

    q4 = nc.declare_dram_parameter("q4", [2, 128, 2, S], F8E4, isOutput=False)
    k4 = nc.declare_dram_parameter("k4", [2, 128, 2, S], F8E4, isOutput=False)
    v4 = nc.declare_dram_parameter("v4", [NP, 128, NT, 66], BF16,
                                   isOutput=False)
    # cols 0:128 = Id2 (half identities), 128:256 = tri2 (MASKC where
    # k+64j < t); identical content on partitions 0-63 and 64-127
    cst = nc.declare_dram_parameter("cst", [128, 2, 256], F8E5, isOutput=False)
    # unnormalized [s, feature|denom] output per s-chunk
    out4 = nc.declare_dram_parameter("out4", [NP, 2, 128, 8, 65], BF16,
                                     isOutput=True)

    # small groups (<= 4 blocks = 512 cols = one PSUM bank); each group's
    # exp is handled by ONE engine (greedy weighted assignment) so every
    # score tile has a single reader and cross-engine sync per unit of
    # work stays low.  6 sc banks in flight + 2 o_ps = 8 PSUM banks.
    # Group size per engine: ACT/DVE 4 blocks, Pool 3 (its per-column rate
    # is lowest and its loop through the PE static order is critical).
    BLOCKS = {0: _blocks_of(0), 1: _blocks_of(1)}

    with tile.TileContext(nc) as tc:
        with (
            tc.tile_pool(name="persist", bufs=1) as pp,
            tc.tile_pool(name="attn", bufs=6) as at,
            tc.tile_pool(name="epi", bufs=2) as ep,
            tc.tile_pool(name="psc", bufs=2, space="PSUM") as psc,
            tc.tile_pool(name="pso", bufs=1, space="PSUM") as pso,
        ):
            QT = [None, None]
            KT = [None, None]
            VT = [None] * NP

            # zero bias column for the ACT exps
            zc = pp.tile([128, 1], F32, tag="zc")
            nc.vector.memset(zc, 0.0)

            cst_t = pp.tile([128, 2, 256], F8E5, tag="cst")

            # PE warm-up chain during the initial DMAs
            wsb = pp.tile([128, 512], BF16, tag="wsb")
            nc.vector.memset(wsb, 0.125)
            wps = psc.tile([128, 128 * int(os.environ.get("GSZ", "8,8").split(",")[0])],
                           F32, tag="sc", name="wps",
                           bufs=int(os.environ.get("SCBUFS", "3")))
            for _ in range(5):
                nc.tensor.matmul(wps[0:2, 0:512], wsb[:, 0:2], wsb,
                                 start=True, stop=True,
                                 skip_group_check=True)

            # upfront loads; the first QK group's operands are split
            # fine-grained so compute can start after ~1 us of DMA
            nc.sync.dma_start(out=cst_t, in_=cst[:, :, :])
            for i in range(2):
                QT[i] = pp.tile([128, 2, S], F8E4, tag=f"q{i}", name="q_t")
                KT[i] = pp.tile([128, 2, S], F8E4, tag=f"k{i}", name="k_t")
            for p in range(NP):
                VT[p] = pp.tile([128, NT, 66], BF16, tag=f"vt{p}",
                                name="vt")
            # pair-0/1 criticals first, split fine; tile-1 and later V
            # tiles trail (needed only ~1/4 into the stream)
            nc.sync.dma_start(out=QT[0][:, :, 0:512],
                              in_=q4[0][:, :, 0:512])
            nc.gpsimd.dma_start(out=KT[0][:, :, 0:256],
                                in_=k4[0][:, :, 0:256])
            nc.sync.dma_start(out=QT[0][:, :, 512:1024],
                              in_=q4[0][:, :, 512:1024])
            nc.gpsimd.dma_start(out=KT[0][:, :, 256:512],
                                in_=k4[0][:, :, 256:512])
            nc.sync.dma_start(out=VT[0][:, 0:4], in_=v4[0][:, 0:4])
            nc.gpsimd.dma_start(out=KT[0][:, :, 512:1024],
                                in_=k4[0][:, :, 512:1024])
            nc.sync.dma_start(out=QT[0][:, :, 1024:2048],
                              in_=q4[0][:, :, 1024:2048])
            nc.gpsimd.dma_start(out=KT[0][:, :, 1024:2048],
                                in_=k4[0][:, :, 1024:2048])
            nc.sync.dma_start(out=VT[0][:, 4:NT], in_=v4[0][:, 4:NT])
            nc.sync.dma_start(out=QT[1], in_=q4[1][:, :, :])
            nc.gpsimd.dma_start(out=KT[1], in_=k4[1][:, :, :])
            nc.sync.dma_start(out=VT[1], in_=v4[1])
            nc.gpsimd.dma_start(out=VT[2], in_=v4[2])
            nc.sync.dma_start(out=VT[3], in_=v4[3])

            # build groups + engine assignment together: greedy weighted
            # per-group engine choice; the chosen engine determines the
            # group size taken from the block stream
            ECOST = tuple(float(x) for x in os.environ.get(
                "ECOST", "1000,1150").split(","))
            GSZ = tuple(int(x) for x in os.environ.get(
                "GSZ", "8,8").split(","))
            _t = [0.0] * len(ECOST)
            GROUPS = {}     # (p, h) -> list of block-lists
            ENG_M = {}      # (p, h) -> list of engine ids
            for p in range(NP):
                for h in range(2):
                    gl, el = [], []
                    pos = 0
                    blocks_h = BLOCKS[h]
                    n_g = 0
                    while pos < len(blocks_h):
                        e = min(range(len(ECOST)),
                                key=lambda i: _t[i] + ECOST[i])
                        take = min(GSZ[e], len(blocks_h) - pos)
                        if p == 0 and h == 0 and n_g < 4:
                            take = min(4, take)   # prime the pipeline
                        n_g += 1
                        gl.append(blocks_h[pos:pos + take])
                        el.append(e)
                        _t[e] += ECOST[e] * take / GSZ[e]
                        pos += take
                    GROUPS[(p, h)] = gl
                    ENG_M[(p, h)] = el
            steps = [(p, h, g)
                     for p in range(NP) for h in range(2)
                     for g in range(len(GROUPS[(p, h)]))]
            ENG = []
            for p, h, g in steps:
                ENG.append(ENG_M[(p, h)][g])
            SC = {}
            OPS = {}

            SCTAG = ("sc", "sc")
            PTTAG = ("pta", "ptd")

            def emit_qk(idx):
                p, h, g = steps[idx]
                ti = p // 2
                pb = 64 * (p % 2)          # base partition of this pair
                blocks = GROUPS[(p, h)][g]
                w_all = 128 * len(blocks)
                sc = psc.tile([128, 128 * max(GSZ)], F32,
                              tag=SCTAG[ENG[idx]], name="sc",
                              bufs=int(os.environ.get("SCBUFS", "3")))
                SC[idx] = sc
                started_banks = set()
                with tc.high_priority():
                    # contiguous same-ii runs -> DoubleRow QK matmuls.
                    # PSUM zero-region is a whole 2KB bank: only the FIRST
                    # write into each bank may use start=True (it zeroes
                    # the entire bank), and chunks are 256-aligned in tile
                    # coords so no chunk crosses a bank boundary.
                    i = 0
                    while i < len(blocks):
                        j = i
                        ii = blocks[i][0]
                        while (j + 1 < len(blocks)
                               and blocks[j + 1][0] == ii
                               and blocks[j + 1][1] == blocks[j][1] + 1):
                            j += 1
                        s0 = 128 * blocks[i][1]
                        x0 = 128 * i
                        w = 128 * (j - i + 1)
                        u0 = x0
                        while u0 < x0 + w:
                            u1 = min(x0 + w, (u0 // 256 + 1) * 256)
                            bank = u0 // 512
                            st = bank not in started_banks
                            started_banks.add(bank)
                            nc.tensor.matmul(
                                sc[:, u0:u1],
                                KT[ti][pb:pb + 64, :,
                                       128 * ii:128 * ii + 128],
                                QT[ti][pb:pb + 64, :,
                                       s0 + u0 - x0:s0 + u1 - x0],
                                start=st, stop=True, perf_mode=DR,
                                skip_group_check=True)
                            u0 = u1
                        i = j + 1
                    # diagonal pre-mask accumulate: tri2^T @ Id2 = MASKC
                    # where s < t, added onto the diagonal blocks
                    for bi, (ii, c) in enumerate(blocks):
                        if ii == c:
                            nc.tensor.matmul(
                                sc[:, 128 * bi:128 * bi + 128],
                                cst_t[pb:pb + 64, :, 128:256],
                                cst_t[pb:pb + 64, :, 0:128],
                                start=False, stop=True, perf_mode=DR,
                                skip_group_check=True)

            LOOKAHEAD = int(os.environ.get("LOOKAHEAD", "6"))
            CONV_DELAY = 5
            pending = []      # (due_idx, p, h, q0, rot)
            dma_left = {}     # (p, h) -> remaining converts before DMA
            conv_rot = [0]

            def emit_conv(p, h, q0, rot):
                o_ps = OPS[(p, h)]
                us = OPS[(p, h, "us")]
                if rot == 0:
                    nc.scalar.copy(us[:, q0:q0 + 4, :],
                                   o_ps[:, q0:q0 + 4, 0:65])
                else:
                    nc.vector.tensor_scalar(
                        us[:, q0:q0 + 4, :], o_ps[:, q0:q0 + 4, 0:65],
                        1.0, None, OP.mult)
                dma_left[(p, h)] -= 1
                if p == NP - 1 and h == 1:
                    # final half: ship each quad as soon as it converts
                    nc.sync.dma_start(out=out4[p, h][:, q0:q0 + 4, :],
                                      in_=us[:, q0:q0 + 4, :])
                elif dma_left[(p, h)] == 0:
                    nc.sync.dma_start(out=out4[p, h],
                                      in_=OPS[(p, h, "us")])

            def flush_pending(idx):
                while pending and pending[0][0] <= idx:
                    _, p_, h_, q0_, rot_ = pending.pop(0)
                    emit_conv(p_, h_, q0_, rot_)

            PV_DELAY = int(os.environ.get("PV_DELAY", "2"))
            pv_pending = []   # (due_idx, idx, p, h, g, pt)

            def emit_pv(idx, p, h, g, pt):
                o_ps = OPS[(p, h)]
                blocks = GROUPS[(p, h)][g]
                for bi, (ii, c) in enumerate(blocks):
                    cc = c - 8 * h
                    # start=True only for the bank's first write (chunk 0
                    # and chunk 4 at ii=0): a start=True write zeroes the
                    # whole 2KB bank, so the other chunks accumulate onto
                    # the pending-zeroed bank instead
                    nc.tensor.matmul(
                        o_ps[:, cc, 0:65],
                        pt[:, 128 * bi:128 * bi + 128].bitcast(BF16),
                        VT[p][:, ii, 0:65],
                        start=(ii == 0 and cc % 4 == 0), stop=(ii == c),
                        skip_group_check=True)
                    if ii == c and cc in (3, 7):
                        rot = conv_rot[0]
                        conv_rot[0] = (conv_rot[0] + 1) % 2
                        if cc == 7:
                            dly = int(os.environ.get("CONV_DB", "0"))
                        else:
                            dly = int(os.environ.get("CONV_DA", "3"))
                        if dly == 0:
                            emit_conv(p, h, cc - 3, rot)
                        else:
                            pending.append((idx + dly, p, h, cc - 3, rot))

            def flush_pv(idx):
                while pv_pending and pv_pending[0][0] <= idx:
                    _, i_, p_, h_, g_, pt_ = pv_pending.pop(0)
                    emit_pv(i_, p_, h_, g_, pt_)
                    flush_pending(idx)

            for j in range(LOOKAHEAD):
                emit_qk(j)
            for idx, step in enumerate(steps):
                p, h, g = step
                if g == 0:
                    OPS[(p, h)] = pso.tile([128, 8, 128], F32, tag="ops",
                                           name="ops")
                    US = ep.tile([128, 8, 65], BF16, tag="us")
                    OPS[(p, h, "us")] = US
                    dma_left[(p, h)] = 2
                flush_pv(idx)
                if idx + LOOKAHEAD < len(steps):
                    emit_qk(idx + LOOKAHEAD)
                blocks = GROUPS[(p, h)][g]
                n = 128 * len(blocks)
                sc = SC.pop(idx)
                e = ENG[idx]

                pt = at.tile([128, 128 * max(GSZ)], I16, tag=PTTAG[e],
                             bufs=int(os.environ.get("PTBUFS", "4")))
                if e == 0:
                    if idx == 0:
                        # split the very first exp so it can start right
                        # after the first QK block
                        for n0 in range(0, n, 256):
                            n1 = min(n, n0 + 256)
                            nc.scalar.activation(
                                pt[:, n0:n1].bitcast(BF16), sc[:, n0:n1],
                                AF.Exp, bias=zc[:, 0:1],
                                scale=float(1.0 / SCALE))
                    else:
                        nc.scalar.activation(
                            pt[:, 0:n].bitcast(BF16), sc[:, 0:n], AF.Exp,
                            bias=zc[:, 0:1], scale=float(1.0 / SCALE))
                else:
                    nc.vector.tensor_scalar(
                        pt[:, 0:n], sc[:, 0:n],
                        float(A_SCH), B_SCH, OP.mult, OP.add)

                # transposed PV per 128-col block, deferred so the PE
                # never stalls on a not-yet-finished exp (in-order queue);
                # the final half inlines so the tail drains promptly
                if PV_DELAY == 0 or (p == NP - 1 and h == 1):
                    # drain ALL earlier deferred PVs/converts first so
                    # o_ps emission order (and thus dep order) is correct
                    flush_pv(len(steps) + PV_DELAY)
                    flush_pending(len(steps) + PV_DELAY + CONV_DELAY)
                    emit_pv(idx, p, h, g, pt)
                    flush_pending(idx)
                else:
                    pv_pending.append((idx + PV_DELAY, idx, p, h, g, pt))
            flush_pv(len(steps) + PV_DELAY)
            flush_pending(len(steps) + PV_DELAY + CONV_DELAY)

    nc.finalize()
    return nc


def _get_nc(key=None):
    if "nc" not in _CACHE:
        _CACHE["nc"] = _build_nc()
    return _CACHE["nc"]


def kernel(x, positions, w_q, b_q, w_k, b_k, w_v, b_v, w_out, b_out,
           _trace=False, _trace_kwargs=None):
    x = np.ascontiguousarray(np.asarray(x), np.float32)
    positions = np.asarray(positions, np.float64)
    w_q = np.asarray(w_q); b_q = np.asarray(b_q)
    w_k = np.asarray(w_k); b_k = np.asarray(b_k)
    w_v = np.asarray(w_v); b_v = np.asarray(b_v)
    w_out = np.asarray(w_out); b_out = np.asarray(b_out)

    # phases (radians, reduced mod 2pi in f64 for accuracy)
    t = np.mod(positions * PHI, 2 * np.pi).astype(np.float32)   # [S]
    cq = (1.0 / (1.0 + np.abs(w_q))).astype(np.float32)         # [H,DH]
    ck = (1.0 / (1.0 + np.abs(w_k))).astype(np.float32)
    cv = (1.0 / (1.0 + np.abs(w_v))).astype(np.float32)
    wsc = (1.0 / (1.0 + np.abs(w_out.astype(np.float64)))
           ).astype(np.float32).reshape(H, DH)
    bo = (b_out.astype(np.float32) + np.float32(PI / 4)).reshape(H, DH)

    nc = _get_nc()

    # constant mask operands: Id2[k,j,s] = [k+64j == s],
    # tri2[k,j,t] = MASKC * [k+64j < t]
    cst_np = np.zeros((128, 2, 256), F8W)
    kk = np.arange(64)
    for j in range(2):
        row = kk + 64 * j
        idm = np.zeros((64, 128), np.float32)
        idm[kk, kk + 64 * j] = 1.0
        tri = np.where(row[:, None] < np.arange(128)[None, :], MASKC, 0.0)
        for pb in (0, 64):
            cst_np[pb:pb + 64, j, 0:128] = idm.astype(F8W)
            cst_np[pb:pb + 64, j, 128:256] = tri.astype(F8W)

    in_maps = []
    pair_bh = []
    for core in range(8):
        b = core // 4
        h0 = 4 * (core % 4)
        pairs = [(b, h0 + j) for j in range(NP)]
        pair_bh.append(pairs)
        q4 = np.empty((2, 128, 2, S), F8)
        k4 = np.empty((2, 128, 2, S), F8)
        v4 = np.zeros((NP, 128, NT, 66), BF)
        for j, (b_, h_) in enumerate(pairs):
            xs = x[b_, :, h_ * DH:(h_ + 1) * DH]                # [S, DH]
            thq = xs * cq[h_][None, :] + b_q[h_][None, :] + t[:, None]
            thk = xs * ck[h_][None, :] + b_k[h_][None, :]
            thv = xs * cv[h_][None, :] + b_v[h_][None, :] + t[:, None]
            ti, pb = j // 2, 64 * (j % 2)
            q4[ti, pb:pb + 64, 0, :] = np.cos(thq).T
            q4[ti, pb:pb + 64, 1, :] = np.sin(thq).T
            k4[ti, pb:pb + 64, 0, :] = np.cos(thk).T
            k4[ti, pb:pb + 64, 1, :] = np.sin(thk).T
            vv = (np.cos(thv) + np.sin(thv)).reshape(NT, 128, DH)
            v4[j, :, :, 0:DH] = vv.transpose(1, 0, 2)
            v4[j, :, :, DH] = 1.0
        in_maps.append(dict(q4=q4, k4=k4, v4=v4, cst=cst_np))

    res = run_bass_kernel_spmd(nc, in_maps, list(range(8)),
                               trace=_trace, **(_trace_kwargs or {}))

    # host epilogue: divide by the denominator column and apply
    # sqrt(2) * sin(u/(1+|w_out|) + b_out + pi/4)
    rt2 = np.float32(math.sqrt(2.0))
    out = np.empty((B, S, D), np.float32)
    for core in range(8):
        o4 = res.results[core]["out4"].astype(np.float32)  # [NP,2,128,8,65]
        for j, (b_, h_) in enumerate(pair_bh[core]):
            u = o4[j, :, :, :, 0:DH] / o4[j, :, :, :, DH:DH + 1]
            arg = u * wsc[h_][None, None, None, :] + bo[h_][None, None, None, :]
            r = rt2 * np.sin(arg)                  # [2, 128, 8, 64]
            out[b_, :, h_ * DH:(h_ + 1) * DH] = (
                r.transpose(0, 2, 1, 3).reshape(S, DH))
    if _trace:
        return out, res
    return out
